# revision 1
# baseline (speedup 1.0000x reference)
"""Trainium2 Bass kernel for nn_DensityVQC (batched 2-qubit VQC Z-expectation).

Algebra
-------
The reference builds rho_b = conj(psi_b) psi_b^T (note: transpose of the
standard density matrix), evolves rho' = U rho U^dag and returns
tr(rho' Z0) with Z0 = diag(1,1,-1,-1).  This collapses to a per-row
quadratic form: with V = conj(U) (the transposed-rho convention flips the
conjugation) and phi = V psi,

    out_b = |phi_0|^2 + |phi_1|^2 - |phi_2|^2 - |phi_3|^2
          = 2 * || C psi_b ||^2 - ||psi_b||^2        (C = V[0:2, :], U unitary)
          = || A r_b + B m_b ||^2 - 1                (inputs are unit-norm)

with real 4x4 matrices A = sqrt(2)*[Re C; Im C], B = sqrt(2)*[-Im C; Re C].
So the device kernel is: per batch row (r, m in R^4), compute w = A r + B m,
then out = sum(w^2) - 1.  No [B,4,4] density matrices are ever materialized.

Device mapping (per core, pure data parallel over 8 cores)
----------------------------------------------------------
Host-side marshalling (the sharding step) reshapes each core's slice into
component-major layout [128 = 32 groups x 4 comps, 4096] so the device
needs no transposes; loads are perfectly contiguous plain DMAs.

Per supertile of 512 free columns (16384 batch rows):
  1. PE: phi = blkdiag32(A^T)^T . rt + blkdiag32(B^T)^T . mt  (two
     accumulating float32r matmuls at full PE rate, moving operands are
     DMA-resident input slices)
  2. ACT Square: S = phi^2 -> SBUF (f32r)
  3. PE: one reduce matmul (stationary = group-sum pattern [128,32],
     moving = S) -> out32 [32, 512] in PSUM
  4. ACT/DVE copy with -1 bias -> resident [32, 4096] output tile
A dummy-matmul burst during the load window warms the PE HAM clock-gate so
the real matmuls run at 2.4 GHz.  The host un-permutes the [32, 4096]
output tile back to batch order (pure data marshalling).
"""

import sys
import numpy as np

if "/opt/trn_rl_repo" not in sys.path:
    sys.path.insert(0, "/opt/trn_rl_repo")

import concourse.bass as bass
import concourse.tile as tile
from concourse import bacc, mybir
from concourse import bass_utils
from concourse.tile_rust import add_dep_helper

N_CORES = 8
BSZ = 1_048_576
BC = BSZ // N_CORES            # 131072 rows per core
NCOL = BC // 32                # 4096 component-major free columns
N_ST = NCOL // 512             # 8 supertiles
N_WARM = 0                     # HAM warm-up (disabled: loads are the bottleneck and PE re-throttles between chunks anyway)
F32 = mybir.dt.float32
F32R = mybir.dt.float32r
N_LAYERS = 6


def _circuit_unitary(ry, rz):
    """4x4 circuit unitary, float64 mirror of reference._circuit_unitary."""
    ry = np.asarray(ry, dtype=np.float64)
    rz = np.asarray(rz, dtype=np.float64)
    cnot = np.array(
        [[1, 0, 0, 0], [0, 1, 0, 0], [0, 0, 0, 1], [0, 0, 1, 0]],
        dtype=np.complex128,
    )

    def _ry(th):
        c, s = np.cos(th / 2), np.sin(th / 2)
        return np.array([[c, -s], [s, c]], dtype=np.complex128)

    def _rz(th):
        return np.diag([np.exp(-0.5j * th), np.exp(0.5j * th)])

    u = np.eye(4, dtype=np.complex128)
    for l in range(ry.shape[0]):
        ry_full = np.kron(_ry(ry[l, 0]), _ry(ry[l, 1]))
        rz_full = np.kron(_rz(rz[l, 0]), _rz(rz[l, 1]))
        u = cnot @ (rz_full @ (ry_full @ u))
    return u


def _host_consts(ry_params, rz_params):
    u = _circuit_unitary(ry_params, rz_params)
    c = np.conj(u)[0:2, :]
    a = np.sqrt(2.0) * np.vstack([c.real, c.imag])     # 4x4, w = A r + B m
    b = np.sqrt(2.0) * np.vstack([-c.imag, c.real])
    eye32 = np.eye(32, dtype=np.float32)
    # lhsT[k=4g+c, m=4g+j] = A[j, c]  ->  block_diag of A.T
    ablk = np.kron(eye32, a.T.astype(np.float32)).astype(np.float32)
    bblk = np.kron(eye32, b.T.astype(np.float32)).astype(np.float32)
    zsum = np.kron(eye32, np.ones((4, 1), dtype=np.float32)).astype(np.float32)
    # Four partition-shifted reduce patterns: zq[k, 32q+g] = zsum[k, g].
    # Supertile st (q = st%4) accumulates its group-sums into partitions
    # [32q, 32q+32) of a shared full-width PSUM bank.
    zqs = []
    for q in range(4):
        zq = np.zeros((128, 128), dtype=np.float32)
        zq[:, 32 * q : 32 * (q + 1)] = zsum
        zqs.append(zq)
    return ablk, bblk, zqs


# Any fixed permutation of the 4096 32-row blocks works (the host inverts
# it); identity keeps the input marshalling a pure reshape+transpose.
def _to_component_major(x):
    """x [BC,4] f32 -> [128, NCOL] f32: column N holds batch rows
    [32N, 32N+32) x 4 comps on the 128 partitions."""
    return np.ascontiguousarray(x.reshape(NCOL, 128).T)


def _from_out32(y):
    """y [2, 128, 512] -> [BC]: value for supertile st = 4h+q, col n, group g
    lives at y[h, 32q+g, n]; batch b = 16384*st + 32n + g."""
    return np.ascontiguousarray(
        y.reshape(2, 4, 32, 512).transpose(0, 1, 3, 2)
    ).reshape(-1)


def _build_program():
    nc = bacc.Bacc("TRN2", target_bir_lowering=False, debug=False)
    # Consts ride as leading columns of the input tensors so no separate
    # DMA (descgen + completion receipt) delays the first data chunk.
    rt_d = nc.dram_tensor("rt", [128, 512 + NCOL], F32R, kind="ExternalInput")
    mt_d = nc.dram_tensor("mt", [128, 256 + NCOL], F32R, kind="ExternalInput")
    out_d = nc.dram_tensor("out", [2, 128, 512], F32, kind="ExternalOutput")

    out_lo_d = out_d.ap()[0]
    out_hi_d = out_d.ap()[1]

    with tile.TileContext(nc) as tc:
        with (
            tc.tile_pool(name="const", bufs=1) as cpool,
            tc.tile_pool(name="io", bufs=1) as iopool,
            tc.tile_pool(name="work", bufs=4) as wpool,
            tc.tile_pool(name="psum", bufs=3, space=bass.MemorySpace.PSUM) as ppool,
        ):
            rt_t = iopool.tile([128, 512 + NCOL], F32R, name="rt_t")
            mt_t = iopool.tile([128, 256 + NCOL], F32R, name="mt_t")
            zq = [rt_t[:, 128 * q : 128 * (q + 1)] for q in range(4)]
            ablk = mt_t[:, 0:128]
            bblk = mt_t[:, 128:256]
            half = NCOL // 2
            # Full 128-partition output tiles (a 32-partition tile only uses
            # 1/4 of the SBUF DMA ports): supertile st lands on partitions
            # [32*(st%4), +32) at columns [512*(st//4), +512).
            out_lo = iopool.tile([128, 512], F32, name="out_lo")
            out_hi = iopool.tile([128, 512], F32, name="out_hi")

            # Small first/last data chunks (early start, short tail); the
            # first chunk of each tensor also carries its consts.
            rb = [0, 1024, 2048, 3072, 4096, 512 + NCOL]
            mb = [0, 768, 1792, 2816, 3840, 256 + NCOL]
            prev_r, prev_m = None, None
            for q in range(len(rb) - 1):
                rqs = bass.ds(rb[q], rb[q + 1] - rb[q])
                mqs = bass.ds(mb[q], mb[q + 1] - mb[q])
                r_dma = nc.sync.dma_start(rt_t[:, rqs], rt_d.ap()[:, rqs])
                m_dma = nc.scalar.dma_start(mt_t[:, mqs], mt_d.ap()[:, mqs])
                # Ordering-only edges: keep the scheduler from reordering
                # chunks (queues are FIFO; a late-scheduled early chunk
                # stalls consumers).
                if prev_r is not None:
                    add_dep_helper(r_dma.ins, prev_r.ins, sync=False, reason="q")
                    add_dep_helper(m_dma.ins, prev_m.ins, sync=False, reason="q")
                prev_r, prev_m = r_dma, m_dma

            # HAM warm-up: dense dummy matmuls on the const tile keep the PE
            # busy through the load window so real matmuls run at 2.4 GHz.
            # Two alternating PSUM buffers so warm-up matmuls pipeline
            # back-to-back (a single buffer serializes on fill-after-drain).
            warm_a = ppool.tile([128, 512], F32, name="warm_a", bufs=1)
            warm_b = ppool.tile([128, 512], F32, name="warm_b", bufs=1)
            for w in range(N_WARM):
                nc.tensor.matmul((warm_a if w % 2 else warm_b)[:], ablk, cstz[:])

            for st in range(N_ST):
                cs = bass.ts(st, 512)
                phi = ppool.tile([128, 512], F32, name="phi", bufs=4)
                nc.tensor.matmul(
                    phi[:], ablk, rt_t[:, 512 + 512 * st : 512 + 512 * (st + 1)],
                    start=True, stop=False,
                )
                nc.tensor.matmul(
                    phi[:], bblk, mt_t[:, 256 + 512 * st : 256 + 512 * (st + 1)],
                    start=False, stop=True,
                )

                s_sb = wpool.tile([128, 512], F32R, name="s_sb")
                nc.scalar.activation(
                    s_sb[:], phi[:], mybir.ActivationFunctionType.Square
                )

                q = st % 4
                if q == 0:
                    ored = ppool.tile([128, 512], F32, name="ored", bufs=2)
                nc.tensor.matmul(
                    ored[:], zq[q], s_sb[:], start=(q == 0), stop=(q == 3)
                )

                if q == 3:
                    # One full-width PSUM -> SBUF copy (with the -1 fold)
                    # per half; alternate engines.
                    out_t = out_lo if st < 4 else out_hi
                    if st < 4:
                        nc.scalar.activation(
                            out_t[:],
                            ored[:],
                            mybir.ActivationFunctionType.Copy,
                            bias=-1.0,
                        )
                    else:
                        nc.vector.tensor_scalar_add(out_t[:], ored[:], -1.0)

                if st == 3:
                    nc.sync.dma_start(out_lo_d, out_lo[:])
            nc.sync.dma_start(out_hi_d, out_hi[:])
    nc.compile()
    return nc


_PROG_CACHE = None


def _get_program():
    global _PROG_CACHE
    if _PROG_CACHE is None:
        _PROG_CACHE = _build_program()
    return _PROG_CACHE


def _run(ry_params, rz_params, states_real, states_imag, **hw_kwargs):
    ablk, bblk, zqs = _host_consts(ry_params, rz_params)
    csta = np.concatenate([ablk, bblk], axis=1).astype(np.float32)
    cstz = np.concatenate(zqs, axis=1).astype(np.float32)
    states_real = np.ascontiguousarray(states_real, dtype=np.float32)
    states_imag = np.ascontiguousarray(states_imag, dtype=np.float32)
    in_maps = []
    for k in range(N_CORES):
        sl = slice(k * BC, (k + 1) * BC)
        in_maps.append(
            {
                "rt": np.concatenate(
                    [cstz, _to_component_major(states_real[sl])], axis=1
                ),
                "mt": np.concatenate(
                    [csta, _to_component_major(states_imag[sl])], axis=1
                ),
            }
        )
    nc = _get_program()
    res = bass_utils.run_bass_kernel_spmd(
        nc, in_maps, core_ids=list(range(N_CORES)), **hw_kwargs
    )
    out = np.concatenate(
        [_from_out32(res.results[k]["out"]) for k in range(N_CORES)]
    ).astype(np.float32)
    return out, res


def kernel(ry_params, rz_params, states_real, states_imag):
    out, _ = _run(ry_params, rz_params, states_real, states_imag)
    return out



# revision 14
# speedup vs baseline: 1.0474x; 1.0474x over previous
"""Trainium2 Bass kernel for nn_DensityVQC (batched 2-qubit VQC Z-expectation).

Algebra
-------
The reference builds rho_b = conj(psi_b) psi_b^T (note: transpose of the
standard density matrix), evolves rho' = U rho U^dag and returns
tr(rho' Z0) with Z0 = diag(1,1,-1,-1).  This collapses to a per-row
quadratic form: with V = conj(U) (the transposed-rho convention flips the
conjugation) and phi = V psi,

    out_b = |phi_0|^2 + |phi_1|^2 - |phi_2|^2 - |phi_3|^2
          = 2 * || C psi_b ||^2 - ||psi_b||^2        (C = V[0:2, :], U unitary)
          = || A r_b + B m_b ||^2 - 1                (inputs are unit-norm)

with real 4x4 matrices A = sqrt(2)*[Re C; Im C], B = sqrt(2)*[-Im C; Re C].
So the device kernel is: per batch row (r, m in R^4), compute w = A r + B m,
then out = sum(w^2) - 1.

Device mapping (per core, pure data parallel over 8 cores)
----------------------------------------------------------
Host-side marshalling interleaves r and m into ONE component-major tensor:
partition p = 8*g + c holds component c (r0..r3,m0..m3) of state-group g;
column n carries 16 states.  One [128,64] block-diagonal stationary W
(W[8g+c, 4g+j] = P[j,c], P = [A|B]) computes all four phi components of 16
groups in a single full-rate float32r matmul per 512-column supertile --
no PSUM accumulation pair and only one weight set for the whole projection.

PE array column-tiling (tile_position) packs results without padded weight
variants: the even supertile of a pair lands on PSUM rows 0:64, the odd on
rows 64:128 of the same bank, so one Square per PAIR ([128,512]) instead of
two.  The [128,32] group-sum stationary reduces a squared pair into rows
32q:32q+32 of a shared output bank (tile_position=(0,32q)), so one
PSUM->SBUF copy (with the -1 fold) covers four pairs.  Squares alternate
ACT/DVE; output stores ride the GpSimd SWDGE queue so the mid-stream store
never queues behind input chunks on the two HWDGE input rings.
"""

import sys
import numpy as np

if "/opt/trn_rl_repo" not in sys.path:
    sys.path.insert(0, "/opt/trn_rl_repo")

import concourse.bass as bass
import concourse.tile as tile
from concourse import bacc, mybir
from concourse import bass_utils
from concourse.tile_rust import add_dep_helper

N_CORES = 8
BSZ = 1_048_576
BC = BSZ // N_CORES            # 131072 rows per core
NCOL = BC // 16                # 8192 component-major free columns
CCOLS = 96                     # DMA'd const cols: Wproj [128,64] + zred [128,32]
CPAD = 416                     # on-SBUF zero-padded const block width
N_PAIRS = 8                    # pairs of 512-col supertiles
N_WARM = 10                    # PE clock-ramp warm-up matmuls
# Input chunk sizes (columns): big early for stream rate, small late so the
# last pair's compute chain starts as soon as possible.
CHUNKS = [2048, 2048, 2048, 1024, 512, 512]
F32 = mybir.dt.float32
F32R = mybir.dt.float32r
N_LAYERS = 6


def _circuit_unitary(ry, rz):
    """4x4 circuit unitary, float64 mirror of reference._circuit_unitary."""
    ry = np.asarray(ry, dtype=np.float64)
    rz = np.asarray(rz, dtype=np.float64)
    cnot = np.array(
        [[1, 0, 0, 0], [0, 1, 0, 0], [0, 0, 0, 1], [0, 0, 1, 0]],
        dtype=np.complex128,
    )

    def _ry(th):
        c, s = np.cos(th / 2), np.sin(th / 2)
        return np.array([[c, -s], [s, c]], dtype=np.complex128)

    def _rz(th):
        return np.diag([np.exp(-0.5j * th), np.exp(0.5j * th)])

    u = np.eye(4, dtype=np.complex128)
    for l in range(ry.shape[0]):
        ry_full = np.kron(_ry(ry[l, 0]), _ry(ry[l, 1]))
        rz_full = np.kron(_rz(rz[l, 0]), _rz(rz[l, 1]))
        u = cnot @ (rz_full @ (ry_full @ u))
    return u


def _host_consts(ry_params, rz_params):
    u = _circuit_unitary(ry_params, rz_params)
    c = np.conj(u)[0:2, :]
    a = np.sqrt(2.0) * np.vstack([c.real, c.imag])     # 4x4, w = A r + B m
    b = np.sqrt(2.0) * np.vstack([-c.imag, c.real])
    p = np.concatenate([a, b], axis=1).astype(np.float32)   # [4, 8]
    # Wproj[8g+c, 4g+j] = P[j, c]: the one projection stationary.  Matmul
    # PSUM writes cannot target a partition offset (walrus rejects nonzero
    # tile positions), so the odd/even supertile placement uses overlapping
    # 128-col slices of one zero-padded block: [Z64 | Wproj | Z64] gives
    # Wlo = cst[:, 64:192] = [Wproj | 0] (rows 0:64) and
    # Whi = cst[:, 0:128]  = [0 | Wproj] (rows 64:128), accumulated in PSUM.
    wproj = np.zeros((128, 64), dtype=np.float32)
    for g in range(16):
        wproj[8 * g : 8 * g + 8, 4 * g : 4 * g + 4] = p.T
    # zred[64h+4g+j, 16h+g] = 1.0: per-state sum of the 4 squared components
    # of a squared pair (even sup on rows 0:64, odd on 64:128).  Padded the
    # same way: [Z96 | zred | Z96]; zq[q] = slice [288-32q : 416-32q] puts
    # the group-sums on output rows 32q:32q+32.
    zred = np.zeros((128, 32), dtype=np.float32)
    for h in range(2):
        for g in range(16):
            zred[64 * h + 4 * g : 64 * h + 4 * g + 4, 16 * h + g] = 1.0
    return np.concatenate([wproj, zred], axis=1)       # [128, 96]


def _to_component_major(u8):
    """u8 [BC,8] f32 -> [128, NCOL]: tile[8g+c, n] = u8[16n+g, c]."""
    return np.ascontiguousarray(
        u8.reshape(NCOL, 16, 8).transpose(1, 2, 0).reshape(128, NCOL)
    )


def _from_out(y):
    """y [128, 1024] -> [BC].  Row 32q+16h+g, col 512B+n holds the value for
    state b = 16*(512*st + n) + g with st = 2*(4B+q)+h."""
    return np.ascontiguousarray(
        y.reshape(4, 2, 16, 2, 512).transpose(3, 0, 1, 4, 2)
    ).reshape(BC)


def _build_program():
    nc = bacc.Bacc("TRN2", target_bir_lowering=False, debug=False)
    ut_d = nc.dram_tensor("ut", [128, CCOLS + NCOL], F32R, kind="ExternalInput")
    out_d = nc.dram_tensor("out", [128, 1024], F32, kind="ExternalOutput")
    # The Activation HWDGE queue is never used (all loads ride the SP ring,
    # stores the SWDGE ring).  Dropping its declaration shrinks the runtime's
    # per-ring teardown drain (16 fewer rings to poll at NEFF exit).
    nc.m.queues = [q for q in nc.m.queues if q.name != "qActDynamicHW"]
    nc.hwdge_engines = type(nc.hwdge_engines)([mybir.EngineType.SP])

    with tile.TileContext(nc) as tc:
        with (
            tc.tile_pool(name="io", bufs=1) as iopool,
            tc.tile_pool(name="work", bufs=4) as wpool,
            tc.tile_pool(name="psum", bufs=1, space=bass.MemorySpace.PSUM) as ppool,
        ):
            # SBUF layout: [0:CPAD) zero-padded const block, [CPAD:) data.
            ut_t = iopool.tile([128, CPAD + NCOL], F32R, name="ut_t")
            out_sb = iopool.tile([128, 1024], F32, name="out_sb")
            wlo = ut_t[:, 64:192]     # [Wproj | 0] -> phi rows 0:64
            whi = ut_t[:, 0:128]      # [0 | Wproj] -> phi rows 64:128
            zq = [ut_t[:, 288 - 32 * q : 416 - 32 * q] for q in range(4)]

            # Zero padding built on device (3 memsets on the idle Pool
            # engine) so the const DMA carries only the 96 real columns.
            # Bitcast to uint32: the Memset ISA op rejects float32r APs.
            nc.gpsimd.memset(ut_t[:, 0:64].bitcast(mybir.dt.uint32), 0)
            nc.gpsimd.memset(ut_t[:, 128:288].bitcast(mybir.dt.uint32), 0)
            nc.gpsimd.memset(ut_t[:, 320:416].bitcast(mybir.dt.uint32), 0)

            # --- input DMA, all on the SP HWDGE ring (one InstDMACopy
            # already spreads across all 16 SDMA engines; one ring keeps
            # chunk arrival strictly in-order).
            dma_prev = {0: None, 2: None}

            def load(dst0, src0, n):
                d = nc.sync.dma_start(ut_t[:, dst0:dst0 + n],
                                      ut_d.ap()[:, src0:src0 + n])
                if dma_prev[0] is not None:
                    add_dep_helper(d.ins, dma_prev[0].ins, sync=False,
                                   reason="q")
                dma_prev[0] = d

            load(64, 0, 64)            # Wproj -> cst cols 64:128
            load(288, 64, 32)          # zred  -> cst cols 288:320
            src = CCOLS
            dst = CPAD
            for n in CHUNKS:
                load(dst, src, n)
                src += n
                dst += n

            # --- PE clock-ramp warm-up on the const columns.
            warm = ppool.tile([128, 512], F32, name="warm", bufs=1)
            for w in range(N_WARM):
                nc.tensor.matmul(warm[:, 0:CPAD], wlo, ut_t[:, 0:CPAD],
                                 start=True, stop=True)

            phis = [None] * N_PAIRS
            sqs = [None] * N_PAIRS
            obank = [None, None]

            def proj(t):
                phi = ppool.tile([128, 512], F32, name="phi", bufs=4)
                phis[t] = phi
                ce = CPAD + 1024 * t
                nc.tensor.matmul(phi[:], wlo, ut_t[:, ce:ce + 512],
                                 start=True, stop=False)
                nc.tensor.matmul(phi[:], whi, ut_t[:, ce + 512:ce + 1024],
                                 start=False, stop=True)

            def square(t):
                # ACT only: DVE TensorTensor cannot read two PSUM operands.
                s_t = wpool.tile([128, 512], F32R, name="s")
                sqs[t] = s_t
                nc.scalar.activation(
                    s_t[:], phis[t][:], mybir.ActivationFunctionType.Square
                )

            def reduce(t):
                q, b = t % 4, t // 4
                if q == 0:
                    obank[b] = ppool.tile([128, 512], F32, name="ob", bufs=2)
                nc.tensor.matmul(obank[b][:], zq[q], sqs[t][:],
                                 start=(q == 0), stop=(q == 3))

            def drain(b):
                # PSUM -> SBUF with the -1 fold (DVE, keeping ACT free for
                # squares), then store on the SWDGE queue (keeps the
                # mid-stream store off the input ring).
                cs = bass.ds(512 * b, 512)
                nc.vector.tensor_scalar_add(out_sb[:, cs], obank[b][:], -1.0)
                d = nc.gpsimd.dma_start(out_d.ap()[:, cs], out_sb[:, cs])
                if dma_prev[2] is not None:
                    add_dep_helper(d.ins, dma_prev[2].ins, sync=False,
                                   reason="q")
                dma_prev[2] = d

            # Two-deep stagger: reduce(t-2) is emitted after proj(t), so by
            # the time the in-order PE reaches it, square(t-2) has long
            # finished and the PE never stalls on the ACT engine.
            proj(0)
            proj(1)
            square(0)
            square(1)
            for t in range(2, N_PAIRS):
                proj(t)
                reduce(t - 2)
                square(t)
                if t - 2 == 3:
                    drain(0)
            reduce(N_PAIRS - 2)
            reduce(N_PAIRS - 1)
            drain(1)
    nc.compile()
    return nc


_PROG_CACHE = None


def _get_program():
    global _PROG_CACHE
    if _PROG_CACHE is None:
        _PROG_CACHE = _build_program()
    return _PROG_CACHE


def _run(ry_params, rz_params, states_real, states_imag, **hw_kwargs):
    consts = _host_consts(ry_params, rz_params)
    states_real = np.ascontiguousarray(states_real, dtype=np.float32)
    states_imag = np.ascontiguousarray(states_imag, dtype=np.float32)
    in_maps = []
    for k in range(N_CORES):
        sl = slice(k * BC, (k + 1) * BC)
        u8 = np.concatenate([states_real[sl], states_imag[sl]], axis=1)
        in_maps.append(
            {"ut": np.concatenate([consts, _to_component_major(u8)], axis=1)}
        )
    nc = _get_program()
    res = bass_utils.run_bass_kernel_spmd(
        nc, in_maps, core_ids=list(range(N_CORES)), **hw_kwargs
    )
    out = np.concatenate(
        [_from_out(res.results[k]["out"]) for k in range(N_CORES)]
    ).astype(np.float32)
    return out, res


def kernel(ry_params, rz_params, states_real, states_imag):
    out, _ = _run(ry_params, rz_params, states_real, states_imag)
    return out
